# revision 18
# baseline (speedup 1.0000x reference)
"""Cadzow update (batched rank-K truncation + Toeplitz averaging) on 8 trn2 cores.

Data-parallel over 128 matrices (16/core). Per matrix (256x256):
  A = w1@Sp + w2@Tp + w4*Tp + w3*T
    -> computed elementwise as c1*Sp + c2*Tp + w3*(T - Tp)  (w1,w2 diagonal,
       w3 == -w4; verified on host, general fallback otherwise)
  Tpnew = rank-K(A) via subspace ladder + small Rayleigh-Ritz:
    K1 (device): G = A^T A (fp32r), squarings G2(scaled), G4, G8(bf16);
      16-dim subspace ladder on G8, seeded with G4 columns; per rung a
      batched (8 matrices per [16,128] packed tile) trace-normalized
      quintic Newton-Schulz orthogonalization; fp32 polish; outputs
      V (256x16), B1 = A V, Gh = V^T G4 V, and diag-sums of Sp.
    host bridge: 16x16 eigh -> top-K projector P; C = B1 P; diag-sums of
      Tpnew = sum_l xcorr(C_l, V_l) via FFT; avg row of 2*Tpnew - Sp.
    K2 (device): Tpnew = C (V)^T from CT/VT inputs; Spnew = Sp - Tpnew + toep
      with toep built on-chip from the avg row (DMA window read + PE flip).
"""
import os
import numpy as np
from contextlib import ExitStack

os.environ.pop("BASS_TRACE", None)  # ntff hook unavailable under this axon env

import concourse.bass as bass
import concourse.bacc as bacc
import concourse.mybir as mybir
from concourse import tile
from concourse.bass_utils import run_bass_kernel_spmd

F32 = mybir.dt.float32
F32R = mybir.dt.float32r
BF16 = mybir.dt.bfloat16
AL = mybir.AluOpType
AF = mybir.ActivationFunctionType

N_CORES = 8
B_FULL = 128
BPC = B_FULL // N_CORES     # 16 matrices per core
R = 256
H = 128
LA = 16                     # ladder width
NG = 8                      # matrices per ladder group
MUO = (3.4445, -4.7750, 2.0315)
NSQ = (1.875, -1.25, 0.375)
G2_SCALE = 2.0 ** -21

# ladder config (sim-tuned)
N_RUNGS = 8
MUON_STEPS = 3
POLISH_STEPS = 4

SHEAR_N = 512 * 257         # bf16 elems per shear region


def _ld256(nc, dst, src):
    """DRAM (256, 256) -> SBUF [128, 512] (row halves side by side), 1 DMA."""
    nc.sync.dma_start(out=dst[:, :].rearrange("p (h j) -> p h j", h=2),
                      in_=src.rearrange("(h p) j -> p h j", p=H))


def _st256(nc, dst, src):
    """SBUF [128, 512] -> DRAM (256, 256), 1 DMA."""
    nc.sync.dma_start(out=dst.rearrange("(h p) j -> p h j", p=H),
                      in_=src[:, :].rearrange("p (h j) -> p h j", h=2))


class EvacRR:
    """Round-robin PSUM->SBUF copy across DVE and ACT engines."""
    def __init__(self, nc):
        self.nc = nc
        self.i = 0

    def copy(self, out, in_, scale=None):
        eng = (self.nc.vector, self.nc.scalar)[self.i % 2]
        self.i += 1
        if scale is None:
            if eng is self.nc.vector:
                eng.tensor_copy(out, in_)
            else:
                eng.copy(out, in_)
        else:
            if eng is self.nc.vector:
                self.nc.vector.tensor_scalar_mul(out, in_, float(scale))
            else:
                eng.mul(out, in_, float(scale))


def build_k1(c1, c2, bpc=BPC, n_rungs=N_RUNGS, muon_steps=MUON_STEPS,
             polish_steps=POLISH_STEPS, do_shear=True):
    nc = bacc.Bacc("TRN2", target_bir_lowering=False)
    sp_d = nc.dram_tensor("sp", [bpc, R, R], F32, kind="ExternalInput")
    tp_d = nc.dram_tensor("tp", [bpc, R, R], F32, kind="ExternalInput")
    t_d = nc.dram_tensor("t", [bpc, R, R], F32, kind="ExternalInput")
    w3_d = nc.dram_tensor("w3", [R, R], F32, kind="ExternalInput")
    identf_d = nc.dram_tensor("identf", [H, H], F32, kind="ExternalInput")
    idp_d = nc.dram_tensor("idp", [LA, NG * LA], F32, kind="ExternalInput")
    v_out = nc.dram_tensor("v_out", [2, H, NG * 2 * LA], F32, kind="ExternalOutput")
    b1_out = nc.dram_tensor("b1_out", [2, H, NG * 2 * LA], F32, kind="ExternalOutput")
    gh_out = nc.dram_tensor("gh_out", [2, LA, NG * LA], F32, kind="ExternalOutput")
    ds_out = nc.dram_tensor("ds_out", [bpc, 511], F32, kind="ExternalOutput")
    scr_d = nc.dram_tensor("scr", [bpc, SHEAR_N], BF16)

    with tile.TileContext(nc) as tc, ExitStack() as ctx:
        ctx.enter_context(nc.allow_low_precision(reason="fp32r feeds PE; rounding is intentional"))
        cpool = ctx.enter_context(tc.tile_pool(name="consts", bufs=1))
        inpool = ctx.enter_context(tc.tile_pool(name="inp", bufs=2))
        tpool = ctx.enter_context(tc.tile_pool(name="trans", bufs=2))
        keep = ctx.enter_context(tc.tile_pool(name="keep", bufs=1))
        lpool = ctx.enter_context(tc.tile_pool(name="lad", bufs=2))
        spool = ctx.enter_context(tc.tile_pool(name="small", bufs=3))
        pbig = ctx.enter_context(tc.tile_pool(name="pbig", bufs=3, space="PSUM"))
        pmid = ctx.enter_context(tc.tile_pool(name="pmid", bufs=2, space="PSUM"))
        psml = ctx.enter_context(tc.tile_pool(name="psml", bufs=2, space="PSUM"))
        ev = EvacRR(nc)

        w3 = cpool.tile([H, 2 * R], F32)
        _ld256(nc, w3, w3_d)
        identf = cpool.tile([H, H], F32)
        nc.sync.dma_start(out=identf[:, :], in_=identf_d[:, :])
        identb = cpool.tile([H, H], BF16)
        nc.vector.tensor_copy(identb[:, :], identf[:, :])
        identfr = cpool.tile([H, H], F32R)
        nc.vector.tensor_copy(identfr[:, :], identf[:, :])
        ones16f = cpool.tile([LA, 1], F32)
        nc.any.memset(ones16f[:, :], 1.0)
        ones16 = cpool.tile([LA, 1], F32R)
        nc.vector.tensor_copy(ones16[:, :], ones16f[:, :])
        ones1x16f = cpool.tile([1, LA], F32)
        nc.any.memset(ones1x16f[:, :], 1.0)
        ones1x16 = cpool.tile([1, LA], F32R)
        nc.vector.tensor_copy(ones1x16[:, :], ones1x16f[:, :])
        onescol = cpool.tile([H, 1], BF16)
        nc.any.memset(onescol[:, :], 1.0)
        # IDP: 8 tiled I16 blocks [16, 128] (host-provided)
        idp = cpool.tile([LA, NG * LA], F32)
        nc.sync.dma_start(out=idp[:, :], in_=idp_d[:, :])
        # aI tiles for NS: MUO[0]*I16 and NSQ[0]*I16 tiled 8x
        aeye_mu = cpool.tile([LA, NG * LA], F32)
        nc.vector.tensor_scalar_mul(aeye_mu[:, :], idp[:, :], float(MUO[0]))
        aeye_ns = cpool.tile([LA, NG * LA], F32)
        nc.vector.tensor_scalar_mul(aeye_ns[:, :], idp[:, :], float(NSQ[0]))
        # shear staging [128, 1024]: data cols 0:256 and 512:768, rest zero
        stg = cpool.tile([H, 1024], BF16)
        nc.any.memset(stg[:, :], 0.0)

        g8s, g4s, ats = [], [], []
        for b in range(bpc):
            sp_t = inpool.tile([H, 2 * R], F32, tag="sp")
            tp_t = inpool.tile([H, 2 * R], F32, tag="tp")
            t_t = inpool.tile([H, 2 * R], F32, tag="t")
            _ld256(nc, sp_t, sp_d[b])
            _ld256(nc, tp_t, tp_d[b])
            _ld256(nc, t_t, t_d[b])

            # A = c1*Sp + c2*Tp + w3*(T - Tp)
            d_t = tpool.tile([H, 2 * R], F32, tag="d")
            nc.gpsimd.tensor_tensor(out=d_t[:, :], in0=t_t[:, :], in1=tp_t[:, :],
                                    op=AL.subtract)
            x_t = tpool.tile([H, 2 * R], F32, tag="x")
            nc.gpsimd.tensor_tensor(out=x_t[:, :], in0=w3[:, :], in1=d_t[:, :],
                                    op=AL.mult)
            a_t = tpool.tile([H, 2 * R], F32R, tag="a")
            nc.vector.scalar_tensor_tensor(out=a_t[:, :], in0=sp_t[:, :],
                                           scalar=float(c1), in1=x_t[:, :],
                                           op0=AL.mult, op1=AL.add)
            nc.vector.scalar_tensor_tensor(out=a_t[:, :], in0=tp_t[:, :],
                                           scalar=float(c2), in1=a_t[:, :],
                                           op0=AL.mult, op1=AL.add)

            if do_shear:
                # diag-sums of Sp via bf16 shear scratch
                nc.scalar.copy(stg[:, 0:R], sp_t[:, 0:R])
                nc.scalar.copy(stg[:, 512:512 + R], sp_t[:, R:2 * R])
                # zero head [0,255) once per region (tiny)
                nc.sync.dma_start(
                    out=scr_d[b][0:255].rearrange("(p f) -> p f", p=1),
                    in_=stg[0:1, 256:511])
                for hh in range(2):
                    dst = scr_d[b][255 + 511 * H * hh: 255 + 511 * H * hh + 511 * H]
                    nc.sync.dma_start(out=dst.rearrange("(p f) -> p f", p=H),
                                      in_=stg[:, 512 * hh: 512 * hh + 511])
                psds = psml.tile([1, 511], F32, tag="sml")
                for hh in range(2):
                    shm = tpool.tile([H, 511], BF16, tag=f"shm{hh}")
                    src = scr_d[b][512 * H * hh: 512 * H * hh + 512 * H]
                    nc.sync.dma_start(
                        out=shm[:, :],
                        in_=src.rearrange("(p f) -> p f", p=H)[:, 0:511])
                    nc.tensor.matmul(psds[:, :], onescol[:, :], shm[:, :],
                                     start=(hh == 0), stop=(hh == 1))
                dsr = spool.tile([1, 511], F32, tag="dsr")
                nc.scalar.copy(dsr[:, :], psds[:, :])
                nc.sync.dma_start(
                    out=ds_out[b].rearrange("(p f) -> p f", p=1), in_=dsr[:, :])

            # AT via 4 PE transposes
            at_t = keep.tile([H, 2 * R], F32R, tag=f"at{b}")
            for i in range(2):
                for j in range(2):
                    pst = pbig.tile([H, H], F32R, tag="big")
                    nc.tensor.transpose(pst[:, :], a_t[:, R * i + H * j: R * i + H * j + H],
                                        identfr[:, :])
                    ev.copy(at_t[:, R * j + H * i: R * j + H * i + H], pst[:, :])

            # G = A^T A (fp32r)
            g_t = tpool.tile([H, 2 * R], F32R, tag="g")
            for mh in range(2):
                ps = pbig.tile([H, R], F32, tag="big")
                for kh in range(2):
                    nc.tensor.matmul(
                        ps[:, :],
                        a_t[:, R * kh + H * mh: R * kh + H * mh + H],
                        a_t[:, R * kh: R * kh + R],
                        start=(kh == 0), stop=(kh == 1))
                ev.copy(g_t[:, R * mh: R * mh + R], ps[:, :])
            # G2 = (G G) * 2^-21
            g2_t = tpool.tile([H, 2 * R], F32R, tag="g2")
            for mh in range(2):
                ps = pbig.tile([H, R], F32, tag="big")
                for kh in range(2):
                    nc.tensor.matmul(
                        ps[:, :],
                        g_t[:, R * kh + H * mh: R * kh + H * mh + H],
                        g_t[:, R * kh: R * kh + R],
                        start=(kh == 0), stop=(kh == 1))
                ev.copy(g2_t[:, R * mh: R * mh + R], ps[:, :], scale=G2_SCALE)
            # G4 = G2 G2 (keep, f32; used for seed + RR)
            g4_t = keep.tile([H, 2 * R], F32R, tag=f"g4_{b}")
            for mh in range(2):
                ps = pbig.tile([H, R], F32, tag="big")
                for kh in range(2):
                    nc.tensor.matmul(
                        ps[:, :],
                        g2_t[:, R * kh + H * mh: R * kh + H * mh + H],
                        g2_t[:, R * kh: R * kh + R],
                        start=(kh == 0), stop=(kh == 1))
                ev.copy(g4_t[:, R * mh: R * mh + R], ps[:, :])
            # G8 = G4 G4 (keep, bf16 for the ladder)
            g8_t = keep.tile([H, 2 * R], BF16, tag=f"g8_{b}")
            for mh in range(2):
                ps = pbig.tile([H, R], F32, tag="big")
                for kh in range(2):
                    nc.tensor.matmul(
                        ps[:, :],
                        g4_t[:, R * kh + H * mh: R * kh + H * mh + H],
                        g4_t[:, R * kh: R * kh + R],
                        start=(kh == 0), stop=(kh == 1))
                ev.copy(g8_t[:, R * mh: R * mh + R], ps[:, :])
            g8s.append(g8_t)
            g4s.append(g4_t)
            ats.append(at_t)

        # ---- ladder: 2 groups x 8 matrices, V packed [128, 8*32] bf16 ----
        n_grp = (bpc + NG - 1) // NG
        vgs = []
        for g in range(n_grp):
            vg = keep.tile([H, NG * 2 * LA], BF16, tag=f"vg{g}")
            # seed: V_k = G4 columns 0..15 (power-4-applied coordinate seed)
            for k in range(NG):
                b = g * NG + k
                for hh in range(2):
                    nc.vector.tensor_copy(
                        vg[:, 32 * k + LA * hh: 32 * k + LA * hh + LA],
                        g4s[b][:, R * hh: R * hh + LA])
            vgs.append(vg)

        def ns_smalls(mg_f32, coef, steps, aeye, dt=BF16):
            """Packed trace-normalized quintic NS on [16, 128] (8 blocks).
            Returns Ct tile (bf16) WITHOUT the 1/sqrt(tr) factor, plus RRB
            broadcast tile holding 1/sqrt(tr) per block."""
            a_c, b_c, c_c = coef
            W = NG * LA
            # diag extract + per-block traces
            md = spool.tile([LA, W], F32R, tag="md")
            nc.vector.tensor_tensor(out=md[:, :], in0=mg_f32[:, :], in1=idp[:, :],
                                    op=AL.mult)
            psd = psml.tile([1, W], F32, tag="sml")
            nc.tensor.matmul(psd[:, :], ones16[:, :], md[:, :], start=True, stop=True)
            dr = spool.tile([1, W], F32, tag="dr")
            nc.scalar.copy(dr[:, :], psd[:, :])
            tr8 = spool.tile([1, NG], F32, tag="tr8")
            nc.vector.tensor_reduce(
                out=tr8[:, :].unsqueeze(-1),
                in_=dr[:, :].rearrange("p (k f) -> p k f", f=LA),
                axis=mybir.AxisListType.X, op=AL.add)
            nc.vector.tensor_scalar_add(tr8[:, :], tr8[:, :], 1e-30)
            irow = spool.tile([1, 2 * NG], F32R, tag="irow")
            nc.vector.reciprocal(irow[:, 0:NG], tr8[:, :])
            sq = spool.tile([1, NG], F32, tag="sq")
            nc.scalar.activation(sq[:, :], tr8[:, :], AF.Sqrt)
            nc.vector.reciprocal(irow[:, NG:2 * NG], sq[:, :])
            psE = pmid.tile([LA, 2 * W], F32, tag="mid")
            nc.tensor.matmul(
                psE[:, :], ones1x16[:, :],
                irow[:, :].unsqueeze(-1).broadcast_to((1, 2 * NG, LA)),
                start=True, stop=True)
            eb = spool.tile([LA, 2 * W], F32, tag="eb")
            nc.vector.tensor_copy(eb[:, :], psE[:, :])
            trb, rrb = eb[:, 0:W], eb[:, W:2 * W]
            mn = spool.tile([LA, W], dt, tag="mn")
            nc.vector.tensor_tensor(out=mn[:, :], in0=mg_f32[:, :], in1=trb,
                                    op=AL.mult)

            def mm8(lhs, rhs, otag):
                ps = psml.tile([LA, W], F32, tag="sml")
                for k in range(NG):
                    nc.tensor.matmul(ps[:, LA * k: LA * k + LA],
                                     lhs[:, LA * k: LA * k + LA],
                                     rhs[:, LA * k: LA * k + LA],
                                     start=True, stop=True)
                ot = spool.tile([LA, W], dt, tag=otag)
                ev.copy(ot[:, :], ps[:, :])
                return ot

            ct = None
            mcur = mn
            for st in range(steps):
                m2 = mm8(mcur, mcur, "m2")
                cstf = spool.tile([LA, W], F32, tag="cf")
                nc.vector.scalar_tensor_tensor(out=cstf[:, :], in0=mcur[:, :],
                                               scalar=float(b_c), in1=aeye[:, :],
                                               op0=AL.mult, op1=AL.add)
                cst = spool.tile([LA, W], dt, tag="cs")
                nc.vector.scalar_tensor_tensor(out=cst[:, :], in0=m2[:, :],
                                               scalar=float(c_c), in1=cstf[:, :],
                                               op0=AL.mult, op1=AL.add)
                if st < steps - 1:
                    cm = mm8(cst, mcur, "cm")
                    mcur = mm8(cm, cst, "mc")
                ct = cst if ct is None else mm8(ct, cst, "ct")
            # fold rr into ct
            ctf = spool.tile([LA, W], dt, tag="ctf")
            nc.vector.tensor_tensor(out=ctf[:, :], in0=ct[:, :], in1=rrb, op=AL.mult)
            return ctf

        def rung(g, ridx):
            vg = vgs[g]
            W = NG * 2 * LA
            psY = pbig.tile([H, W], F32, tag="big")
            for k in range(NG):
                b = g * NG + k
                for hh in range(2):
                    for ch in range(2):
                        nc.tensor.matmul(
                            psY[:, 32 * k + LA * hh: 32 * k + LA * hh + LA],
                            g8s[b][:, R * ch + H * hh: R * ch + H * hh + H],
                            vg[:, 32 * k + LA * ch: 32 * k + LA * ch + LA],
                            start=(ch == 0), stop=(ch == 1))
            yg = lpool.tile([H, W], BF16, tag=f"yg{g}")
            ev.copy(yg[:, :], psY[:, :])
            # Gram [16, 128]
            psM = psml.tile([LA, NG * LA], F32, tag="sml")
            for k in range(NG):
                for hh in range(2):
                    nc.tensor.matmul(
                        psM[:, LA * k: LA * k + LA],
                        yg[:, 32 * k + LA * hh: 32 * k + LA * hh + LA],
                        yg[:, 32 * k + LA * hh: 32 * k + LA * hh + LA],
                        start=(hh == 0), stop=(hh == 1))
            mg = spool.tile([LA, NG * LA], F32, tag="mg")
            nc.scalar.copy(mg[:, :], psM[:, :])
            ctf = ns_smalls(mg, MUO, muon_steps, aeye_mu)
            # transpose Y + apply Ct
            psA = pbig.tile([H, W], F32, tag="big")
            for k in range(NG):
                ytk = lpool.tile([LA, 2 * H], BF16, tag=f"ytk{g}")
                for hh in range(2):
                    psT = pmid.tile([LA, H], BF16, tag="mid")
                    nc.tensor.transpose(
                        psT[:, :], yg[:, 32 * k + LA * hh: 32 * k + LA * hh + LA],
                        identb[:, :])
                    ev.copy(ytk[:, H * hh: H * hh + H], psT[:, :])
                for hh in range(2):
                    nc.tensor.matmul(
                        psA[:, 32 * k + LA * hh: 32 * k + LA * hh + LA],
                        ytk[:, H * hh: H * hh + H],
                        ctf[:, LA * k: LA * k + LA],
                        start=True, stop=True)
            if ridx == n_rungs - 1:
                vf = keep.tile([H, W], F32, tag=f"vf{g}")
                nc.vector.tensor_copy(vf[:, :], psA[:, :])
                vfs.append(vf)
            else:
                nc.vector.tensor_copy(vg[:, :], psA[:, :])

        vfs = []
        for ridx in range(n_rungs):
            for g in range(n_grp):
                rung(g, ridx)

        # ---- polish in f32 ----
        def polish(g):
            vf = vfs[g]
            W = NG * 2 * LA
            psM = psml.tile([LA, NG * LA], F32, tag="sml")
            for k in range(NG):
                for hh in range(2):
                    nc.tensor.matmul(
                        psM[:, LA * k: LA * k + LA],
                        vf[:, 32 * k + LA * hh: 32 * k + LA * hh + LA],
                        vf[:, 32 * k + LA * hh: 32 * k + LA * hh + LA],
                        start=(hh == 0), stop=(hh == 1))
            mg = spool.tile([LA, NG * LA], F32, tag="pmg")
            nc.scalar.copy(mg[:, :], psM[:, :])
            ctf32 = ns_smalls(mg, NSQ, polish_steps, aeye_ns, dt=F32)
            psA = pbig.tile([H, W], F32, tag="big")
            for k in range(NG):
                vtk = lpool.tile([LA, 2 * H], F32, tag=f"vtk{g}")
                for hh in range(2):
                    psT = pmid.tile([LA, H], F32, tag="mid")
                    nc.tensor.transpose(
                        psT[:, :], vf[:, 32 * k + LA * hh: 32 * k + LA * hh + LA],
                        identf[:, :])
                    ev.copy(vtk[:, H * hh: H * hh + H], psT[:, :])
                for hh in range(2):
                    nc.tensor.matmul(
                        psA[:, 32 * k + LA * hh: 32 * k + LA * hh + LA],
                        vtk[:, H * hh: H * hh + H],
                        ctf32[:, LA * k: LA * k + LA],
                        start=True, stop=True)
            nc.vector.tensor_copy(vf[:, :], psA[:, :])

        for g in range(n_grp):
            polish(g)

        # ---- RR (Gh = V^T G4 V) + B1 = A V + outputs ----
        for g in range(n_grp):
            vf = vfs[g]
            W = NG * 2 * LA
            nc.sync.dma_start(out=v_out[g], in_=vf[:, :])
            vfr = lpool.tile([H, W], F32R, tag=f"vfr{g}")
            nc.vector.tensor_copy(vfr[:, :], vf[:, :])
            # Z = G4 V
            psZ = pbig.tile([H, W], F32, tag="big")
            for k in range(NG):
                b = g * NG + k
                for hh in range(2):
                    for ch in range(2):
                        nc.tensor.matmul(
                            psZ[:, 32 * k + LA * hh: 32 * k + LA * hh + LA],
                            g4s[b][:, R * ch + H * hh: R * ch + H * hh + H],
                            vfr[:, 32 * k + LA * ch: 32 * k + LA * ch + LA],
                            start=(ch == 0), stop=(ch == 1))
            zg = lpool.tile([H, W], F32R, tag=f"zg{g}")
            ev.copy(zg[:, :], psZ[:, :])
            psGh = psml.tile([LA, NG * LA], F32, tag="sml")
            for k in range(NG):
                for hh in range(2):
                    nc.tensor.matmul(
                        psGh[:, LA * k: LA * k + LA],
                        vfr[:, 32 * k + LA * hh: 32 * k + LA * hh + LA],
                        zg[:, 32 * k + LA * hh: 32 * k + LA * hh + LA],
                        start=(hh == 0), stop=(hh == 1))
            ghg = spool.tile([LA, NG * LA], F32, tag="ghg")
            nc.scalar.copy(ghg[:, :], psGh[:, :])
            nc.sync.dma_start(out=gh_out[g], in_=ghg[:, :])
            # B1 = A V  (via AT tiles)
            psB = pbig.tile([H, W], F32, tag="big")
            for k in range(NG):
                b = g * NG + k
                for hh in range(2):
                    for ch in range(2):
                        nc.tensor.matmul(
                            psB[:, 32 * k + LA * hh: 32 * k + LA * hh + LA],
                            ats[b][:, R * ch + H * hh: R * ch + H * hh + H],
                            vfr[:, 32 * k + LA * ch: 32 * k + LA * ch + LA],
                            start=(ch == 0), stop=(ch == 1))
            bg = lpool.tile([H, W], F32, tag=f"bg{g}")
            ev.copy(bg[:, :], psB[:, :])
            nc.sync.dma_start(out=b1_out[g], in_=bg[:, :])
    nc.compile()
    return nc


def build_k2(bpc=BPC):
    nc = bacc.Bacc("TRN2", target_bir_lowering=False)
    sp_d = nc.dram_tensor("sp", [bpc, R, R], F32, kind="ExternalInput")
    ct_d = nc.dram_tensor("ct", [bpc, LA, R], F32R, kind="ExternalInput")
    vt_d = nc.dram_tensor("vt", [bpc, LA, R], F32R, kind="ExternalInput")
    avg_d = nc.dram_tensor("avg", [bpc, 512], BF16, kind="ExternalInput")
    identb_d = nc.dram_tensor("identb", [H, H], F32, kind="ExternalInput")
    tpn_out = nc.dram_tensor("tpn_out", [bpc, R, R], F32, kind="ExternalOutput")
    spn_out = nc.dram_tensor("spn_out", [bpc, R, R], F32, kind="ExternalOutput")

    with tile.TileContext(nc) as tc, ExitStack() as ctx:
        cpool = ctx.enter_context(tc.tile_pool(name="consts", bufs=1))
        inpool = ctx.enter_context(tc.tile_pool(name="inp", bufs=3))
        tpool = ctx.enter_context(tc.tile_pool(name="trans", bufs=3))
        pbig = ctx.enter_context(tc.tile_pool(name="pbig", bufs=4, space="PSUM"))
        ev = EvacRR(nc)

        jf32 = cpool.tile([H, H], F32)
        nc.sync.dma_start(out=jf32[:, :], in_=identb_d[:, :])
        jflip = cpool.tile([H, H], BF16)
        nc.vector.tensor_copy(jflip[:, :], jf32[:, :])

        for b in range(bpc):
            sp_t = inpool.tile([H, 2 * R], F32, tag="sp")
            _ld256(nc, sp_t, sp_d[b])
            ct_t = inpool.tile([LA, R], F32R, tag="ct")
            nc.sync.dma_start(out=ct_t[:, :], in_=ct_d[b])
            vt_t = inpool.tile([LA, R], F32R, tag="vt")
            nc.sync.dma_start(out=vt_t[:, :], in_=vt_d[b])
            # toeplitz windows (flipped partition order) from avg row:
            # tf[u, j] = avg[base + u + j] -- overlapping-window AP
            tfl = [None, None]
            for hh in range(2):
                tf = tpool.tile([H, R], BF16, tag=f"tf{hh}")
                base = 128 if hh == 0 else 0
                src = avg_d[b][base:base + 1]
                win = bass.AP(src.tensor, src.offset, [[1, H], [1, R]])
                nc.sync.dma_start(out=tf[:, :], in_=win)
                tfl[hh] = tf
            # Tpnew halves (natural): lhsT = CT slice, rhs = VT
            for hh in range(2):
                psTp = pbig.tile([H, R], F32, tag="psTp")
                nc.tensor.matmul(psTp[:, :],
                                 ct_t[:, H * hh: H * hh + H],
                                 vt_t[:, :], start=True, stop=True)
                tpn_t = tpool.tile([H, R], F32, tag=f"tpn{hh}")
                nc.scalar.copy(tpn_t[:, :], psTp[:, :])
                nc.sync.dma_start(
                    out=tpn_out[b].rearrange("(h p) j -> h p j", p=H)[hh],
                    in_=tpn_t[:, :])
                # spm = Sp - Tpnew
                spm = tpool.tile([H, R], F32, tag=f"spm{hh}")
                nc.vector.scalar_tensor_tensor(
                    out=spm[:, :], in0=psTp[:, :], scalar=-1.0,
                    in1=sp_t[:, R * hh: R * hh + R], op0=AL.mult, op1=AL.add)
                # toep natural = J @ toep_flipped
                psJ = pbig.tile([H, R], F32, tag="psJ")
                toepb = tpool.tile([H, R], BF16, tag=f"tb{hh}")
                nc.vector.tensor_copy(toepb[:, :], tfl[hh][:, :])
                nc.tensor.matmul(psJ[:, :], jflip[:, :], toepb[:, :],
                                 start=True, stop=True)
                spn = tpool.tile([H, R], F32, tag=f"spn{hh}")
                nc.vector.tensor_tensor(out=spn[:, :], in0=spm[:, :], in1=psJ[:, :],
                                        op=AL.add)
                nc.sync.dma_start(
                    out=spn_out[b].rearrange("(h p) j -> h p j", p=H)[hh],
                    in_=spn[:, :])
    nc.compile()
    return nc


# ---------------- host side ----------------

def _host_consts():
    identf = np.eye(H, dtype=np.float32)
    jflip = identf[::-1].copy()
    counts = (R - np.abs(np.arange(511) - 255)).astype(np.float32)
    return identf, jflip, counts


def _bridge(gh_pk, v_pk, b1_pk, ds_sp, Kv):
    """Host bridge for one core's K1 outputs (packed).
    Returns ct [bpc,16,256], vt [bpc,16,256], avg [bpc,512] bf16."""
    import ml_dtypes
    bpc = BPC
    V = np.zeros((bpc, R, LA), np.float32)
    B1 = np.zeros((bpc, R, LA), np.float32)
    Gh = np.zeros((bpc, LA, LA), np.float32)
    for g in range(2):
        for k in range(NG):
            b = g * NG + k
            V[b, 0:H] = v_pk[g][:, 32 * k: 32 * k + LA]
            V[b, H:R] = v_pk[g][:, 32 * k + LA: 32 * k + 2 * LA]
            B1[b, 0:H] = b1_pk[g][:, 32 * k: 32 * k + LA]
            B1[b, H:R] = b1_pk[g][:, 32 * k + LA: 32 * k + 2 * LA]
            Gh[b] = gh_pk[g][:, LA * k: LA * k + LA]
    Ghs = 0.5 * (Gh + Gh.transpose(0, 2, 1))
    d, q = np.linalg.eigh(Ghs.astype(np.float64))
    qk = q[:, :, ::-1][:, :, :Kv]
    P = np.einsum('blk,bmk->blm', qk, qk).astype(np.float32)
    C = np.einsum('brl,blm->brm', B1, P).astype(np.float32)
    # diag-sums of Tpnew = sum_l xcorr(C_l, V_l), lags -255..255
    n_fft = 512
    Fc = np.fft.rfft(C, n_fft, axis=1)
    Fv = np.fft.rfft(V, n_fft, axis=1)
    cc = np.fft.irfft(np.conj(Fc) * Fv, n_fft, axis=1)  # [b, lag, l]
    cc = cc.sum(axis=2)
    # lag s = j - i in [-(255)..255]; irfft gives lag at index (s mod 512)
    ds_tp = np.zeros((bpc, 511), np.float64)
    ds_tp[:, 255:] = cc[:, 0:256]          # s = 0..255 -> d = 255..510
    ds_tp[:, :255] = cc[:, 257:512]        # s = -255..-1 -> d = 0..254
    counts = (R - np.abs(np.arange(511) - 255)).astype(np.float64)
    avg = (2.0 * ds_tp - ds_sp) / counts
    avgp = np.zeros((bpc, 512), np.float32)
    avgp[:, :511] = avg.astype(np.float32)
    ct = np.ascontiguousarray(C.transpose(0, 2, 1))
    vt = np.ascontiguousarray(V.transpose(0, 2, 1))
    return ct, vt, avgp.astype(ml_dtypes.bfloat16)


def _host_fallback(T, Tp, Sp, w1, w2, w3, w4, Kv):
    f32 = np.float32
    A = (np.einsum('rk,bkc->brc', w1, Sp) + np.einsum('rk,bkc->brc', w2, Tp)
         + w4[None] * Tp + w3[None] * T).astype(f32)
    G = np.einsum('brc,brd->bcd', A, A)
    d, q = np.linalg.eigh(G.astype(np.float64))
    qk = q[:, :, ::-1][:, :, :Kv]
    AV = np.einsum('brc,bcl->brl', A.astype(np.float64), qk)
    Tpnew = np.einsum('brl,bcl->brc', AV, qk).astype(f32)
    m = n = R
    D = m + n - 1
    ii = np.arange(m)[:, None]; jj = np.arange(n)[None, :]
    dd = jj - ii + (m - 1)
    M2 = (2.0 * Tpnew - Sp).astype(f32)
    Z = np.zeros((M2.shape[0], m, D), f32)
    Z[:, ii, dd] = M2
    sums = Z.sum(axis=1)
    counts = (m - np.abs(np.arange(D) - (m - 1))).astype(f32)
    avg = sums / counts
    Spnew = (Sp - Tpnew + avg[:, dd]).astype(f32)
    return (T, Tpnew, Spnew)


LAST_EXEC_NS = [None, None]


def _kernel_device(T, Tp, Sp, w1, w2, w3, w4, Kv):
    global LAST_EXEC_NS
    c1 = float(w1[0, 0])
    c2 = float(w2[0, 0])
    identf, jflip, counts = _host_consts()
    idp = np.tile(np.eye(LA, dtype=np.float32), (1, NG))
    core_ids = list(range(N_CORES))
    nc1 = build_k1(c1, c2)
    in_maps1 = []
    for c in range(N_CORES):
        sl = slice(c * BPC, (c + 1) * BPC)
        in_maps1.append({"sp": Sp[sl], "tp": Tp[sl], "t": T[sl],
                         "w3": w3, "identf": identf, "idp": idp})
    r1 = run_bass_kernel_spmd(nc1, in_maps1, core_ids)
    res1 = r1.results

    in_maps2 = []
    for c in range(N_CORES):
        sl = slice(c * BPC, (c + 1) * BPC)
        gh_pk = res1[c]["gh_out"]
        v_pk = res1[c]["v_out"]
        b1_pk = res1[c]["b1_out"]
        ds_sp = res1[c]["ds_out"].astype(np.float64)
        ct, vt, avgp = _bridge(gh_pk, v_pk, b1_pk, ds_sp, Kv)
        in_maps2.append({"sp": Sp[sl], "ct": ct, "vt": vt, "avg": avgp,
                         "identb": jflip})
    nc2 = build_k2()
    r2 = run_bass_kernel_spmd(nc2, in_maps2, core_ids)
    res2 = r2.results
    LAST_EXEC_NS = [r1.exec_time_ns, r2.exec_time_ns]
    Tpnew = np.concatenate([res2[c]["tpn_out"] for c in range(N_CORES)], axis=0)
    Spnew = np.concatenate([res2[c]["spn_out"] for c in range(N_CORES)], axis=0)
    return (T, Tpnew, Spnew)


def kernel(T, Tp, Sp, w1, w2, w3, w4, K):
    T = np.ascontiguousarray(np.asarray(T, dtype=np.float32))
    Tp = np.ascontiguousarray(np.asarray(Tp, dtype=np.float32))
    Sp = np.ascontiguousarray(np.asarray(Sp, dtype=np.float32))
    w1 = np.asarray(w1, dtype=np.float32); w2 = np.asarray(w2, dtype=np.float32)
    w3 = np.asarray(w3, dtype=np.float32); w4 = np.asarray(w4, dtype=np.float32)
    Kv = int(np.asarray(K))
    structured = (Kv <= LA
                  and np.array_equal(w1, np.diag(np.diag(w1)))
                  and np.array_equal(w2, np.diag(np.diag(w2)))
                  and np.allclose(np.diag(w1), w1[0, 0])
                  and np.allclose(np.diag(w2), w2[0, 0])
                  and np.array_equal(w3, -w4))
    if structured:
        try:
            return _kernel_device(T, Tp, Sp, w1, w2, w3, w4, Kv)
        except Exception:
            import traceback
            traceback.print_exc()
            print("device path failed; falling back to host")
    return _host_fallback(T, Tp, Sp, w1, w2, w3, w4, Kv)


# revision 22
# speedup vs baseline: 1.2009x; 1.2009x over previous
"""Cadzow update (batched rank-K truncation + Toeplitz averaging) on 8 trn2 cores.

Data-parallel over 128 matrices (16/core). Per matrix (256x256):
  A = w1@Sp + w2@Tp + w4*Tp + w3*T
    -> computed elementwise as c1*Sp + c2*Tp + w3*(T - Tp)  (w1,w2 diagonal,
       w3 == -w4; verified on host, general fallback otherwise)
  Tpnew = rank-K(A) via subspace ladder + small Rayleigh-Ritz:
    K1 (device): G = A^T A (fp32r), squarings G2(scaled), G4, G8(bf16);
      16-dim subspace ladder on G8, seeded with G4 columns; per rung a
      batched (8 matrices per [16,128] packed tile) trace-normalized
      quintic Newton-Schulz orthogonalization; fp32 polish; outputs
      V (256x16), B1 = A V, Gh = V^T G4 V, and diag-sums of Sp.
    host bridge: 16x16 eigh -> top-K projector P; C = B1 P; diag-sums of
      Tpnew = sum_l xcorr(C_l, V_l) via FFT; avg row of 2*Tpnew - Sp.
    K2 (device): Tpnew = C (V)^T from CT/VT inputs; Spnew = Sp - Tpnew + toep
      with toep built on-chip from the avg row (DMA window read + PE flip).
"""
import os
import numpy as np
from contextlib import ExitStack

os.environ.pop("BASS_TRACE", None)  # ntff hook unavailable under this axon env

import concourse.bass as bass
import concourse.bacc as bacc
import concourse.mybir as mybir
from concourse import tile
from concourse.bass_utils import run_bass_kernel_spmd

F32 = mybir.dt.float32
F32R = mybir.dt.float32r
BF16 = mybir.dt.bfloat16
AL = mybir.AluOpType
AF = mybir.ActivationFunctionType

N_CORES = 8
B_FULL = 128
BPC = B_FULL // N_CORES     # 16 matrices per core
R = 256
H = 128
LA = 16                     # ladder width
NG = 8                      # matrices per ladder group
MUO = (3.4445, -4.7750, 2.0315)
NSQ = (1.875, -1.25, 0.375)
G2_SCALE = 2.0 ** -21

# ladder config (sim-tuned)
N_RUNGS = 7
MUON_STEPS = 3
POLISH_STEPS = 4

SHEAR_N = 512 * 257         # bf16 elems per shear region


def _ld256(nc, dst, src):
    """DRAM (256, 256) -> SBUF [128, 512] (row halves side by side), 1 DMA."""
    nc.sync.dma_start(out=dst[:, :].rearrange("p (h j) -> p h j", h=2),
                      in_=src.rearrange("(h p) j -> p h j", p=H))


def _st256(nc, dst, src):
    """SBUF [128, 512] -> DRAM (256, 256), 1 DMA."""
    nc.sync.dma_start(out=dst.rearrange("(h p) j -> p h j", p=H),
                      in_=src[:, :].rearrange("p (h j) -> p h j", h=2))


class EvacRR:
    """Round-robin PSUM->SBUF copy across DVE and ACT engines."""
    def __init__(self, nc):
        self.nc = nc
        self.i = 0

    def copy(self, out, in_, scale=None):
        eng = (self.nc.vector, self.nc.scalar)[self.i % 2]
        self.i += 1
        if scale is None:
            if eng is self.nc.vector:
                eng.tensor_copy(out, in_)
            else:
                eng.copy(out, in_)
        else:
            if eng is self.nc.vector:
                self.nc.vector.tensor_scalar_mul(out, in_, float(scale))
            else:
                eng.mul(out, in_, float(scale))


def build_k1(c1, c2, bpc=BPC, n_rungs=N_RUNGS, muon_steps=MUON_STEPS,
             polish_steps=POLISH_STEPS, do_shear=True):
    nc = bacc.Bacc("TRN2", target_bir_lowering=False)
    sp_d = nc.dram_tensor("sp", [bpc, R, R], F32, kind="ExternalInput")
    tp_d = nc.dram_tensor("tp", [bpc, R, R], F32, kind="ExternalInput")
    t_d = nc.dram_tensor("t", [bpc, R, R], F32, kind="ExternalInput")
    w3_d = nc.dram_tensor("w3", [R, R], F32, kind="ExternalInput")
    identf_d = nc.dram_tensor("identf", [H, H], F32, kind="ExternalInput")
    idp_d = nc.dram_tensor("idp", [LA, NG * LA], F32, kind="ExternalInput")
    v_out = nc.dram_tensor("v_out", [2, H, NG * 2 * LA], F32, kind="ExternalOutput")
    b1_out = nc.dram_tensor("b1_out", [2, H, NG * 2 * LA], F32, kind="ExternalOutput")
    gh_out = nc.dram_tensor("gh_out", [2, LA, NG * LA], F32, kind="ExternalOutput")
    ds_out = nc.dram_tensor("ds_out", [bpc, 511], F32, kind="ExternalOutput")
    scr_d = nc.dram_tensor("scr", [bpc, SHEAR_N], BF16)

    with tile.TileContext(nc) as tc, ExitStack() as ctx:
        ctx.enter_context(nc.allow_low_precision(reason="fp32r feeds PE; rounding is intentional"))
        cpool = ctx.enter_context(tc.tile_pool(name="consts", bufs=1))
        inpool = ctx.enter_context(tc.tile_pool(name="inp", bufs=2))
        tpool = ctx.enter_context(tc.tile_pool(name="trans", bufs=2))
        keep = ctx.enter_context(tc.tile_pool(name="keep", bufs=1))
        lpool = ctx.enter_context(tc.tile_pool(name="lad", bufs=2))
        spool = ctx.enter_context(tc.tile_pool(name="small", bufs=3))
        pbig = ctx.enter_context(tc.tile_pool(name="pbig", bufs=4, space="PSUM"))
        pmid = ctx.enter_context(tc.tile_pool(name="pmid", bufs=2, space="PSUM"))
        psml = ctx.enter_context(tc.tile_pool(name="psml", bufs=2, space="PSUM"))
        ev = EvacRR(nc)

        w3 = cpool.tile([H, 2 * R], F32)
        _ld256(nc, w3, w3_d)
        identf = cpool.tile([H, H], F32)
        nc.sync.dma_start(out=identf[:, :], in_=identf_d[:, :])
        identb = cpool.tile([H, H], BF16)
        nc.vector.tensor_copy(identb[:, :], identf[:, :])
        identfr = cpool.tile([H, H], F32R)
        nc.vector.tensor_copy(identfr[:, :], identf[:, :])
        ones16f = cpool.tile([LA, 1], F32)
        nc.any.memset(ones16f[:, :], 1.0)
        ones16 = cpool.tile([LA, 1], F32R)
        nc.vector.tensor_copy(ones16[:, :], ones16f[:, :])
        ones1x16f = cpool.tile([1, LA], F32)
        nc.any.memset(ones1x16f[:, :], 1.0)
        ones1x16 = cpool.tile([1, LA], F32R)
        nc.vector.tensor_copy(ones1x16[:, :], ones1x16f[:, :])
        onescol = cpool.tile([H, 1], BF16)
        nc.any.memset(onescol[:, :], 1.0)
        # IDP: 8 tiled I16 blocks [16, 128] (host-provided)
        idp = cpool.tile([LA, NG * LA], F32)
        nc.sync.dma_start(out=idp[:, :], in_=idp_d[:, :])
        # aI tiles for NS: MUO[0]*I16 and NSQ[0]*I16 tiled 8x
        aeye_mu = cpool.tile([LA, NG * LA], F32)
        nc.vector.tensor_scalar_mul(aeye_mu[:, :], idp[:, :], float(MUO[0]))
        aeye_ns = cpool.tile([LA, NG * LA], F32)
        nc.vector.tensor_scalar_mul(aeye_ns[:, :], idp[:, :], float(NSQ[0]))
        # shear staging [128, 1024]: data cols 0:256 and 512:768, rest zero
        stg = cpool.tile([H, 1024], BF16)
        nc.any.memset(stg[:, :], 0.0)

        # zero all shear-region heads [0,255) in one DMA
        if do_shear:
            nc.scalar.dma_start(
                out=scr_d[0:bpc, 0:255], in_=stg[0:bpc, 256:511])

        CH = 2  # matrices per input-load DMA
        g8s, g4s, ats = [], [], []
        dsacc = None
        for b in range(bpc):
            qq = b % CH
            if qq == 0:
                spc = inpool.tile([H, CH * 2 * R], F32, tag="sp")
                tpc = inpool.tile([H, CH * 2 * R], F32, tag="tp")
                ttc = inpool.tile([H, CH * 2 * R], F32, tag="t")
                for dst, src in ((spc, sp_d), (tpc, tp_d), (ttc, t_d)):
                    nc.sync.dma_start(
                        out=dst[:, :].rearrange("p (q h j) -> p q h j", q=CH, h=2),
                        in_=src[b:b + CH].rearrange("q (h p) j -> p q h j", p=H))
            sp_t = spc[:, 2 * R * qq: 2 * R * (qq + 1)]
            tp_t = tpc[:, 2 * R * qq: 2 * R * (qq + 1)]
            t_t = ttc[:, 2 * R * qq: 2 * R * (qq + 1)]

            # A = c1*Sp + c2*Tp + w3*(T - Tp)
            d_t = tpool.tile([H, 2 * R], F32, tag="d")
            nc.gpsimd.tensor_tensor(out=d_t[:, :], in0=t_t[:, :], in1=tp_t[:, :],
                                    op=AL.subtract)
            x_t = tpool.tile([H, 2 * R], F32, tag="x")
            nc.gpsimd.tensor_tensor(out=x_t[:, :], in0=w3[:, :], in1=d_t[:, :],
                                    op=AL.mult)
            a_t = tpool.tile([H, 2 * R], F32R, tag="a")
            nc.vector.scalar_tensor_tensor(out=a_t[:, :], in0=sp_t[:, :],
                                           scalar=float(c1), in1=x_t[:, :],
                                           op0=AL.mult, op1=AL.add)
            nc.vector.scalar_tensor_tensor(out=a_t[:, :], in0=tp_t[:, :],
                                           scalar=float(c2), in1=a_t[:, :],
                                           op0=AL.mult, op1=AL.add)

            if do_shear:
                # diag-sums of Sp via bf16 shear scratch
                nc.scalar.copy(stg[:, 0:R], sp_t[:, 0:R])
                nc.scalar.copy(stg[:, 512:512 + R], sp_t[:, R:2 * R])
                # merged shear write (both halves, 1 DMA on gpsimd queue)
                dst = scr_d[b][255: 255 + 511 * 2 * H]
                nc.gpsimd.dma_start(
                    out=dst.rearrange("(h p f) -> p h f", h=2, p=H),
                    in_=stg[:, :].rearrange("p (h x) -> p h x", h=2)[:, :, 0:511])
                # merged sheared read (1 DMA on gpsimd queue)
                shm = tpool.tile([H, 2 * 511], BF16, tag="shm")
                src = scr_d[b][0: 512 * 2 * H]
                nc.gpsimd.dma_start(
                    out=shm[:, :].rearrange("p (h f) -> p h f", h=2),
                    in_=src.rearrange("(h p f) -> p h f", h=2, p=H)[:, :, 0:511])
                psds = psml.tile([1, 511], F32, tag="sml")
                for hh in range(2):
                    nc.tensor.matmul(psds[:, :], onescol[:, :],
                                     shm[:, 511 * hh: 511 * hh + 511],
                                     start=(hh == 0), stop=(hh == 1))
                if b % 4 == 0:
                    dsacc = spool.tile([1, 4 * 511], F32, tag="dsacc")
                nc.scalar.copy(dsacc[:, 511 * (b % 4): 511 * (b % 4) + 511],
                               psds[:, :])
                if b % 4 == 3:
                    nc.scalar.dma_start(
                        out=ds_out[b - 3: b + 1].rearrange("q f -> (q f)").unsqueeze(0),
                        in_=dsacc[:, :])

            # AT via 4 PE transposes
            at_t = keep.tile([H, 2 * R], F32R, tag=f"at{b}")
            for i in range(2):
                for j in range(2):
                    pst = pbig.tile([H, H], F32R, tag="big")
                    nc.tensor.transpose(pst[:, :], a_t[:, R * i + H * j: R * i + H * j + H],
                                        identfr[:, :])
                    ev.copy(at_t[:, R * j + H * i: R * j + H * i + H], pst[:, :])

            # G = A^T A (fp32r)
            g_t = tpool.tile([H, 2 * R], F32R, tag="g")
            for mh in range(2):
                ps = pbig.tile([H, R], F32, tag="big")
                for kh in range(2):
                    nc.tensor.matmul(
                        ps[:, :],
                        a_t[:, R * kh + H * mh: R * kh + H * mh + H],
                        a_t[:, R * kh: R * kh + R],
                        start=(kh == 0), stop=(kh == 1))
                ev.copy(g_t[:, R * mh: R * mh + R], ps[:, :])
            # G2 = (G G) * 2^-21
            g2_t = tpool.tile([H, 2 * R], F32R, tag="g2")
            for mh in range(2):
                ps = pbig.tile([H, R], F32, tag="big")
                for kh in range(2):
                    nc.tensor.matmul(
                        ps[:, :],
                        g_t[:, R * kh + H * mh: R * kh + H * mh + H],
                        g_t[:, R * kh: R * kh + R],
                        start=(kh == 0), stop=(kh == 1))
                ev.copy(g2_t[:, R * mh: R * mh + R], ps[:, :], scale=G2_SCALE)
            # G4 = G2 G2 (keep, f32; used for seed + RR)
            g4_t = keep.tile([H, 2 * R], F32R, tag=f"g4_{b}")
            for mh in range(2):
                ps = pbig.tile([H, R], F32, tag="big")
                for kh in range(2):
                    nc.tensor.matmul(
                        ps[:, :],
                        g2_t[:, R * kh + H * mh: R * kh + H * mh + H],
                        g2_t[:, R * kh: R * kh + R],
                        start=(kh == 0), stop=(kh == 1))
                ev.copy(g4_t[:, R * mh: R * mh + R], ps[:, :])
            # G8 = G4 G4 (keep, bf16 for the ladder)
            g8_t = keep.tile([H, 2 * R], BF16, tag=f"g8_{b}")
            for mh in range(2):
                ps = pbig.tile([H, R], F32, tag="big")
                for kh in range(2):
                    nc.tensor.matmul(
                        ps[:, :],
                        g4_t[:, R * kh + H * mh: R * kh + H * mh + H],
                        g4_t[:, R * kh: R * kh + R],
                        start=(kh == 0), stop=(kh == 1))
                ev.copy(g8_t[:, R * mh: R * mh + R], ps[:, :])
            g8s.append(g8_t)
            g4s.append(g4_t)
            ats.append(at_t)

        # ---- ladder: 2 groups x 8 matrices, V packed [128, 8*32] bf16 ----
        n_grp = (bpc + NG - 1) // NG
        vgs = []
        for g in range(n_grp):
            vg = keep.tile([H, NG * 2 * LA], BF16, tag=f"vg{g}")
            # seed: V_k = G4 columns 0..15 (power-4-applied coordinate seed)
            for k in range(NG):
                b = g * NG + k
                for hh in range(2):
                    nc.vector.tensor_copy(
                        vg[:, 32 * k + LA * hh: 32 * k + LA * hh + LA],
                        g4s[b][:, R * hh: R * hh + LA])
            vgs.append(vg)

        def ns_smalls(mg_f32, coef, steps, aeye, dt=BF16):
            """Packed trace-normalized quintic NS on [16, 128] (8 blocks).
            Returns Ct tile (bf16) WITHOUT the 1/sqrt(tr) factor, plus RRB
            broadcast tile holding 1/sqrt(tr) per block."""
            a_c, b_c, c_c = coef
            W = NG * LA
            # diag extract + per-block traces
            md = spool.tile([LA, W], F32R, tag="md")
            nc.vector.tensor_tensor(out=md[:, :], in0=mg_f32[:, :], in1=idp[:, :],
                                    op=AL.mult)
            psd = psml.tile([1, W], F32, tag="sml")
            nc.tensor.matmul(psd[:, :], ones16[:, :], md[:, :], start=True, stop=True)
            dr = spool.tile([1, W], F32, tag="dr")
            nc.scalar.copy(dr[:, :], psd[:, :])
            tr8 = spool.tile([1, NG], F32, tag="tr8")
            nc.vector.tensor_reduce(
                out=tr8[:, :].unsqueeze(-1),
                in_=dr[:, :].rearrange("p (k f) -> p k f", f=LA),
                axis=mybir.AxisListType.X, op=AL.add)
            nc.vector.tensor_scalar_add(tr8[:, :], tr8[:, :], 1e-30)
            irow = spool.tile([1, 2 * NG], F32R, tag="irow")
            nc.vector.reciprocal(irow[:, 0:NG], tr8[:, :])
            sq = spool.tile([1, NG], F32, tag="sq")
            nc.scalar.activation(sq[:, :], tr8[:, :], AF.Sqrt)
            nc.vector.reciprocal(irow[:, NG:2 * NG], sq[:, :])
            psE = pmid.tile([LA, 2 * W], F32, tag="mid")
            nc.tensor.matmul(
                psE[:, :], ones1x16[:, :],
                irow[:, :].unsqueeze(-1).broadcast_to((1, 2 * NG, LA)),
                start=True, stop=True)
            eb = spool.tile([LA, 2 * W], F32, tag="eb")
            nc.vector.tensor_copy(eb[:, :], psE[:, :])
            trb, rrb = eb[:, 0:W], eb[:, W:2 * W]
            mn = spool.tile([LA, W], dt, tag="mn")
            nc.vector.tensor_tensor(out=mn[:, :], in0=mg_f32[:, :], in1=trb,
                                    op=AL.mult)

            def mm8(lhs, rhs, otag):
                ps = psml.tile([LA, W], F32, tag="sml")
                for k in range(NG):
                    nc.tensor.matmul(ps[:, LA * k: LA * k + LA],
                                     lhs[:, LA * k: LA * k + LA],
                                     rhs[:, LA * k: LA * k + LA],
                                     start=True, stop=True)
                ot = spool.tile([LA, W], dt, tag=otag)
                ev.copy(ot[:, :], ps[:, :])
                return ot

            ct = None
            mcur = mn
            for st in range(steps):
                m2 = mm8(mcur, mcur, "m2")
                cstf = spool.tile([LA, W], F32, tag="cf")
                nc.vector.scalar_tensor_tensor(out=cstf[:, :], in0=mcur[:, :],
                                               scalar=float(b_c), in1=aeye[:, :],
                                               op0=AL.mult, op1=AL.add)
                cst = spool.tile([LA, W], dt, tag="cs")
                nc.vector.scalar_tensor_tensor(out=cst[:, :], in0=m2[:, :],
                                               scalar=float(c_c), in1=cstf[:, :],
                                               op0=AL.mult, op1=AL.add)
                if st < steps - 1:
                    cm = mm8(cst, mcur, "cm")
                    mcur = mm8(cm, cst, "mc")
                ct = cst if ct is None else mm8(ct, cst, "ct")
            # fold rr into ct
            ctf = spool.tile([LA, W], dt, tag="ctf")
            nc.vector.tensor_tensor(out=ctf[:, :], in0=ct[:, :], in1=rrb, op=AL.mult)
            return ctf

        def rung(g, ridx):
            vg = vgs[g]
            W = NG * 2 * LA
            psY = pbig.tile([H, W], F32, tag="big")
            for k in range(NG):
                b = g * NG + k
                for hh in range(2):
                    for ch in range(2):
                        nc.tensor.matmul(
                            psY[:, 32 * k + LA * hh: 32 * k + LA * hh + LA],
                            g8s[b][:, R * ch + H * hh: R * ch + H * hh + H],
                            vg[:, 32 * k + LA * ch: 32 * k + LA * ch + LA],
                            start=(ch == 0), stop=(ch == 1))
            yg = lpool.tile([H, W], BF16, tag=f"yg{g}")
            ev.copy(yg[:, :], psY[:, :])
            # Gram [16, 128]
            psM = psml.tile([LA, NG * LA], F32, tag="sml")
            for k in range(NG):
                for hh in range(2):
                    nc.tensor.matmul(
                        psM[:, LA * k: LA * k + LA],
                        yg[:, 32 * k + LA * hh: 32 * k + LA * hh + LA],
                        yg[:, 32 * k + LA * hh: 32 * k + LA * hh + LA],
                        start=(hh == 0), stop=(hh == 1))
            mg = spool.tile([LA, NG * LA], F32, tag="mg")
            nc.scalar.copy(mg[:, :], psM[:, :])
            ctf = ns_smalls(mg, MUO, muon_steps, aeye_mu)
            # transpose Y + apply Ct
            psA = pbig.tile([H, W], F32, tag="big")
            for k in range(NG):
                ytk = lpool.tile([LA, 2 * H], BF16, tag=f"ytk{g}")
                for hh in range(2):
                    psT = pmid.tile([LA, H], BF16, tag="mid")
                    nc.tensor.transpose(
                        psT[:, :], yg[:, 32 * k + LA * hh: 32 * k + LA * hh + LA],
                        identb[:, :])
                    ev.copy(ytk[:, H * hh: H * hh + H], psT[:, :])
                for hh in range(2):
                    nc.tensor.matmul(
                        psA[:, 32 * k + LA * hh: 32 * k + LA * hh + LA],
                        ytk[:, H * hh: H * hh + H],
                        ctf[:, LA * k: LA * k + LA],
                        start=True, stop=True)
            if ridx == n_rungs - 1:
                vf = keep.tile([H, W], F32, tag=f"vf{g}")
                nc.vector.tensor_copy(vf[:, :], psA[:, :])
                vfs.append(vf)
            else:
                nc.vector.tensor_copy(vg[:, :], psA[:, :])

        vfs = []
        for ridx in range(n_rungs):
            for g in range(n_grp):
                rung(g, ridx)

        # ---- polish in f32 ----
        def polish(g):
            vf = vfs[g]
            W = NG * 2 * LA
            psM = psml.tile([LA, NG * LA], F32, tag="sml")
            for k in range(NG):
                for hh in range(2):
                    nc.tensor.matmul(
                        psM[:, LA * k: LA * k + LA],
                        vf[:, 32 * k + LA * hh: 32 * k + LA * hh + LA],
                        vf[:, 32 * k + LA * hh: 32 * k + LA * hh + LA],
                        start=(hh == 0), stop=(hh == 1))
            mg = spool.tile([LA, NG * LA], F32, tag="pmg")
            nc.scalar.copy(mg[:, :], psM[:, :])
            ctf32 = ns_smalls(mg, NSQ, polish_steps, aeye_ns, dt=F32)
            psA = pbig.tile([H, W], F32, tag="big")
            for k in range(NG):
                vtk = lpool.tile([LA, 2 * H], F32, tag=f"vtk{g}")
                for hh in range(2):
                    psT = pmid.tile([LA, H], F32, tag="mid")
                    nc.tensor.transpose(
                        psT[:, :], vf[:, 32 * k + LA * hh: 32 * k + LA * hh + LA],
                        identf[:, :])
                    ev.copy(vtk[:, H * hh: H * hh + H], psT[:, :])
                for hh in range(2):
                    nc.tensor.matmul(
                        psA[:, 32 * k + LA * hh: 32 * k + LA * hh + LA],
                        vtk[:, H * hh: H * hh + H],
                        ctf32[:, LA * k: LA * k + LA],
                        start=True, stop=True)
            nc.vector.tensor_copy(vf[:, :], psA[:, :])

        for g in range(n_grp):
            polish(g)

        # ---- RR (Gh = V^T G4 V) + B1 = A V + outputs ----
        for g in range(n_grp):
            vf = vfs[g]
            W = NG * 2 * LA
            nc.sync.dma_start(out=v_out[g], in_=vf[:, :])
            vfr = lpool.tile([H, W], F32R, tag=f"vfr{g}")
            nc.vector.tensor_copy(vfr[:, :], vf[:, :])
            # Z = G4 V
            psZ = pbig.tile([H, W], F32, tag="big")
            for k in range(NG):
                b = g * NG + k
                for hh in range(2):
                    for ch in range(2):
                        nc.tensor.matmul(
                            psZ[:, 32 * k + LA * hh: 32 * k + LA * hh + LA],
                            g4s[b][:, R * ch + H * hh: R * ch + H * hh + H],
                            vfr[:, 32 * k + LA * ch: 32 * k + LA * ch + LA],
                            start=(ch == 0), stop=(ch == 1))
            zg = lpool.tile([H, W], F32R, tag=f"zg{g}")
            ev.copy(zg[:, :], psZ[:, :])
            psGh = psml.tile([LA, NG * LA], F32, tag="sml")
            for k in range(NG):
                for hh in range(2):
                    nc.tensor.matmul(
                        psGh[:, LA * k: LA * k + LA],
                        vfr[:, 32 * k + LA * hh: 32 * k + LA * hh + LA],
                        zg[:, 32 * k + LA * hh: 32 * k + LA * hh + LA],
                        start=(hh == 0), stop=(hh == 1))
            ghg = spool.tile([LA, NG * LA], F32, tag="ghg")
            nc.scalar.copy(ghg[:, :], psGh[:, :])
            nc.sync.dma_start(out=gh_out[g], in_=ghg[:, :])
            # B1 = A V  (via AT tiles)
            psB = pbig.tile([H, W], F32, tag="big")
            for k in range(NG):
                b = g * NG + k
                for hh in range(2):
                    for ch in range(2):
                        nc.tensor.matmul(
                            psB[:, 32 * k + LA * hh: 32 * k + LA * hh + LA],
                            ats[b][:, R * ch + H * hh: R * ch + H * hh + H],
                            vfr[:, 32 * k + LA * ch: 32 * k + LA * ch + LA],
                            start=(ch == 0), stop=(ch == 1))
            bg = lpool.tile([H, W], F32, tag=f"bg{g}")
            ev.copy(bg[:, :], psB[:, :])
            nc.sync.dma_start(out=b1_out[g], in_=bg[:, :])
    nc.compile()
    return nc


def build_k2(bpc=BPC):
    nc = bacc.Bacc("TRN2", target_bir_lowering=False)
    sp_d = nc.dram_tensor("sp", [bpc, R, R], F32, kind="ExternalInput")
    ct_d = nc.dram_tensor("ct", [bpc, LA, R], F32R, kind="ExternalInput")
    vt_d = nc.dram_tensor("vt", [bpc, LA, R], F32R, kind="ExternalInput")
    avg_d = nc.dram_tensor("avg", [bpc, 512], BF16, kind="ExternalInput")
    identb_d = nc.dram_tensor("identb", [H, H], F32, kind="ExternalInput")
    tpn_out = nc.dram_tensor("tpn_out", [bpc, R, R], F32, kind="ExternalOutput")
    spn_out = nc.dram_tensor("spn_out", [bpc, R, R], F32, kind="ExternalOutput")

    with tile.TileContext(nc) as tc, ExitStack() as ctx:
        cpool = ctx.enter_context(tc.tile_pool(name="consts", bufs=1))
        inpool = ctx.enter_context(tc.tile_pool(name="inp", bufs=3))
        tpool = ctx.enter_context(tc.tile_pool(name="trans", bufs=3))
        pbig = ctx.enter_context(tc.tile_pool(name="pbig", bufs=4, space="PSUM"))
        ev = EvacRR(nc)

        jf32 = cpool.tile([H, H], F32)
        nc.sync.dma_start(out=jf32[:, :], in_=identb_d[:, :])
        jflip = cpool.tile([H, H], BF16)
        nc.vector.tensor_copy(jflip[:, :], jf32[:, :])
        # all matrices' CT and VT in one DMA each: [16, bpc*256]
        ctall = cpool.tile([LA, bpc * R], F32R)
        nc.sync.dma_start(out=ctall[:, :].rearrange("p (b j) -> p b j", b=bpc),
                          in_=ct_d[:].rearrange("b p j -> p b j"))
        vtall = cpool.tile([LA, bpc * R], F32R)
        nc.sync.dma_start(out=vtall[:, :].rearrange("p (b j) -> p b j", b=bpc),
                          in_=vt_d[:].rearrange("b p j -> p b j"))

        for b in range(bpc):
            sp_t = inpool.tile([H, 2 * R], F32, tag="sp")
            _ld256(nc, sp_t, sp_d[b])
            ct_t = ctall[:, R * b: R * (b + 1)]
            vt_t = vtall[:, R * b: R * (b + 1)]
            # toeplitz windows (flipped partition order) from avg row, 1 DMA:
            # tf2 cols 0:256 = h1 window (base 0), cols 256:512 = h0 (base 128)
            tf2 = tpool.tile([H, 2 * R], BF16, tag="tf2")
            src = avg_d[b][0:1]
            win = bass.AP(src.tensor, src.offset, [[1, H], [128, 2], [1, R]])
            nc.scalar.dma_start(out=tf2[:, :].rearrange("p (g j) -> p g j", g=2),
                                in_=win)
            tfl = [tf2[:, R: 2 * R], tf2[:, 0: R]]
            # Tpnew halves (natural): lhsT = CT slice, rhs = VT
            tpn_t = tpool.tile([H, 2 * R], F32, tag="tpn")
            spn_t = tpool.tile([H, 2 * R], F32, tag="spn")
            for hh in range(2):
                psTp = pbig.tile([H, R], F32, tag="psTp")
                nc.tensor.matmul(psTp[:, :],
                                 ct_t[:, H * hh: H * hh + H],
                                 vt_t[:, :], start=True, stop=True)
                nc.scalar.copy(tpn_t[:, R * hh: R * hh + R], psTp[:, :])
                # spm = Sp - Tpnew
                spm = tpool.tile([H, R], F32, tag=f"spm{hh}")
                nc.vector.scalar_tensor_tensor(
                    out=spm[:, :], in0=psTp[:, :], scalar=-1.0,
                    in1=sp_t[:, R * hh: R * hh + R], op0=AL.mult, op1=AL.add)
                # toep natural = J @ toep_flipped
                psJ = pbig.tile([H, R], F32, tag="psJ")
                nc.tensor.matmul(psJ[:, :], jflip[:, :], tfl[hh],
                                 start=True, stop=True)
                nc.vector.tensor_tensor(out=spn_t[:, R * hh: R * hh + R],
                                        in0=spm[:, :], in1=psJ[:, :], op=AL.add)
            _st256(nc, tpn_out[b], tpn_t)
            nc.gpsimd.dma_start(
                out=spn_out[b].rearrange("(h p) j -> p h j", p=H),
                in_=spn_t[:, :].rearrange("p (h j) -> p h j", h=2))
    nc.compile()
    return nc


# ---------------- host side ----------------

def _host_consts():
    identf = np.eye(H, dtype=np.float32)
    jflip = identf[::-1].copy()
    counts = (R - np.abs(np.arange(511) - 255)).astype(np.float32)
    return identf, jflip, counts


def _bridge(gh_pk, v_pk, b1_pk, ds_sp, Kv):
    """Host bridge for one core's K1 outputs (packed).
    Returns ct [bpc,16,256], vt [bpc,16,256], avg [bpc,512] bf16."""
    import ml_dtypes
    bpc = BPC
    V = np.zeros((bpc, R, LA), np.float32)
    B1 = np.zeros((bpc, R, LA), np.float32)
    Gh = np.zeros((bpc, LA, LA), np.float32)
    for g in range(2):
        for k in range(NG):
            b = g * NG + k
            V[b, 0:H] = v_pk[g][:, 32 * k: 32 * k + LA]
            V[b, H:R] = v_pk[g][:, 32 * k + LA: 32 * k + 2 * LA]
            B1[b, 0:H] = b1_pk[g][:, 32 * k: 32 * k + LA]
            B1[b, H:R] = b1_pk[g][:, 32 * k + LA: 32 * k + 2 * LA]
            Gh[b] = gh_pk[g][:, LA * k: LA * k + LA]
    Ghs = 0.5 * (Gh + Gh.transpose(0, 2, 1))
    d, q = np.linalg.eigh(Ghs.astype(np.float64))
    qk = q[:, :, ::-1][:, :, :Kv]
    P = np.einsum('blk,bmk->blm', qk, qk).astype(np.float32)
    C = np.einsum('brl,blm->brm', B1, P).astype(np.float32)
    # diag-sums of Tpnew = sum_l xcorr(C_l, V_l), lags -255..255
    n_fft = 512
    Fc = np.fft.rfft(C, n_fft, axis=1)
    Fv = np.fft.rfft(V, n_fft, axis=1)
    cc = np.fft.irfft(np.conj(Fc) * Fv, n_fft, axis=1)  # [b, lag, l]
    cc = cc.sum(axis=2)
    # lag s = j - i in [-(255)..255]; irfft gives lag at index (s mod 512)
    ds_tp = np.zeros((bpc, 511), np.float64)
    ds_tp[:, 255:] = cc[:, 0:256]          # s = 0..255 -> d = 255..510
    ds_tp[:, :255] = cc[:, 257:512]        # s = -255..-1 -> d = 0..254
    counts = (R - np.abs(np.arange(511) - 255)).astype(np.float64)
    avg = (2.0 * ds_tp - ds_sp) / counts
    avgp = np.zeros((bpc, 512), np.float32)
    avgp[:, :511] = avg.astype(np.float32)
    ct = np.ascontiguousarray(C.transpose(0, 2, 1))
    vt = np.ascontiguousarray(V.transpose(0, 2, 1))
    return ct, vt, avgp.astype(ml_dtypes.bfloat16)


def _host_fallback(T, Tp, Sp, w1, w2, w3, w4, Kv):
    f32 = np.float32
    A = (np.einsum('rk,bkc->brc', w1, Sp) + np.einsum('rk,bkc->brc', w2, Tp)
         + w4[None] * Tp + w3[None] * T).astype(f32)
    G = np.einsum('brc,brd->bcd', A, A)
    d, q = np.linalg.eigh(G.astype(np.float64))
    qk = q[:, :, ::-1][:, :, :Kv]
    AV = np.einsum('brc,bcl->brl', A.astype(np.float64), qk)
    Tpnew = np.einsum('brl,bcl->brc', AV, qk).astype(f32)
    m = n = R
    D = m + n - 1
    ii = np.arange(m)[:, None]; jj = np.arange(n)[None, :]
    dd = jj - ii + (m - 1)
    M2 = (2.0 * Tpnew - Sp).astype(f32)
    Z = np.zeros((M2.shape[0], m, D), f32)
    Z[:, ii, dd] = M2
    sums = Z.sum(axis=1)
    counts = (m - np.abs(np.arange(D) - (m - 1))).astype(f32)
    avg = sums / counts
    Spnew = (Sp - Tpnew + avg[:, dd]).astype(f32)
    return (T, Tpnew, Spnew)


LAST_EXEC_NS = [None, None]


def _kernel_device(T, Tp, Sp, w1, w2, w3, w4, Kv):
    global LAST_EXEC_NS
    c1 = float(w1[0, 0])
    c2 = float(w2[0, 0])
    identf, jflip, counts = _host_consts()
    idp = np.tile(np.eye(LA, dtype=np.float32), (1, NG))
    core_ids = list(range(N_CORES))
    nc1 = build_k1(c1, c2)
    in_maps1 = []
    for c in range(N_CORES):
        sl = slice(c * BPC, (c + 1) * BPC)
        in_maps1.append({"sp": Sp[sl], "tp": Tp[sl], "t": T[sl],
                         "w3": w3, "identf": identf, "idp": idp})
    r1 = run_bass_kernel_spmd(nc1, in_maps1, core_ids)
    res1 = r1.results

    in_maps2 = []
    for c in range(N_CORES):
        sl = slice(c * BPC, (c + 1) * BPC)
        gh_pk = res1[c]["gh_out"]
        v_pk = res1[c]["v_out"]
        b1_pk = res1[c]["b1_out"]
        ds_sp = res1[c]["ds_out"].astype(np.float64)
        ct, vt, avgp = _bridge(gh_pk, v_pk, b1_pk, ds_sp, Kv)
        in_maps2.append({"sp": Sp[sl], "ct": ct, "vt": vt, "avg": avgp,
                         "identb": jflip})
    nc2 = build_k2()
    r2 = run_bass_kernel_spmd(nc2, in_maps2, core_ids)
    res2 = r2.results
    LAST_EXEC_NS = [r1.exec_time_ns, r2.exec_time_ns]
    Tpnew = np.concatenate([res2[c]["tpn_out"] for c in range(N_CORES)], axis=0)
    Spnew = np.concatenate([res2[c]["spn_out"] for c in range(N_CORES)], axis=0)
    return (T, Tpnew, Spnew)


def kernel(T, Tp, Sp, w1, w2, w3, w4, K):
    T = np.ascontiguousarray(np.asarray(T, dtype=np.float32))
    Tp = np.ascontiguousarray(np.asarray(Tp, dtype=np.float32))
    Sp = np.ascontiguousarray(np.asarray(Sp, dtype=np.float32))
    w1 = np.asarray(w1, dtype=np.float32); w2 = np.asarray(w2, dtype=np.float32)
    w3 = np.asarray(w3, dtype=np.float32); w4 = np.asarray(w4, dtype=np.float32)
    Kv = int(np.asarray(K))
    structured = (Kv <= LA
                  and np.array_equal(w1, np.diag(np.diag(w1)))
                  and np.array_equal(w2, np.diag(np.diag(w2)))
                  and np.allclose(np.diag(w1), w1[0, 0])
                  and np.allclose(np.diag(w2), w2[0, 0])
                  and np.array_equal(w3, -w4))
    if structured:
        try:
            return _kernel_device(T, Tp, Sp, w1, w2, w3, w4, Kv)
        except Exception:
            import traceback
            traceback.print_exc()
            print("device path failed; falling back to host")
    return _host_fallback(T, Tp, Sp, w1, w2, w3, w4, Kv)
